# revision 7
# baseline (speedup 1.0000x reference)
"""Depthwise causal conv1d (W=8) with 3 interleaved weight sets, on 8 TRN2 cores.

Reference computes r/o/a = depthwise_causal_conv(x, {rtg,obs,act}_{w,b}) and
interleaves out[:, t] = {r,o,a}[:, t] by t % 3.  Only the t%3-matching third of
each conv is needed, so total work is exactly one conv: for each output t,
out[b,t,h] = sum_k x[b, t-7+k, h] * w_{t%3}[h, k] + b_{t%3}[h].

Strategy (channel-parallel, 96 channels per core, banded-Toeplitz matmul,
two channels packed per stationary matrix):
  - time goes on the PE contraction axis.  For channel pair (ca, cb), one
    [128 x 114] fp16 stationary matrix is block-diagonal: rows 0..63 hold
    ca's banded Toeplitz T[m, p] = w_{p%3}[ca, m-p] (0 <= m-p < 8) over
    output columns 0..56, rows 64..127 hold cb's band over columns 57..113.
    S=57 outputs per block (57 % 3 == 0 keeps the t%3 phase pattern the
    same in every block; window V = S+7 = 64 rows exactly fills half the
    contraction dim).  rhs = [128 x 864] stacks both channels' input
    windows for all 864 = 16 batches x 54 blocks columns, so one weight
    load serves a pair's entire workload; packing halves the dense-w DMA
    vs one channel per 128-row stationary.
  - each pair runs 2 matmuls (columns 0:432, 432:864 — PSUM bank limit is
    512 f32) that share one ldweights; a post-compile pass drops the
    redundant second weight load.
  - PSUM f32; ACT/DVE evict [114 x 2x432] to fp16 with the per-(channel,
    p%3) bias fused via a per-partition bias column (host-precomputed).
  - in-DMAs dispatched from the SP HWDGE, out-DMAs from the ACT HWDGE; big
    contiguous per-partition rows spread across all 16 DMA engines
    (~23 GB/s each), which is the binding resource for this kernel.
  - host pre/post stages the overlapped-window layout (fp16, unit-stride).
fp16 end-to-end rel err ~6e-4.
"""

import os
import numpy as np
from numpy.lib.stride_tricks import as_strided

B, T, H, W = 16, 3072, 768, 8
NCORES = 8
HC = H // NCORES             # 96 channels per core
S = 57                       # outputs per block (multiple of 3)
V = S + W - 1                # 64-row window per channel
NB = 54                      # blocks cover NB*S = 3078 >= T
PADL = W - 1                 # causal left zero-pad
XLEN = S * (NB - 1) + V      # 3085 padded time extent
COLS = B * NB                # 864 rhs columns per channel
HB = 432                     # matmul column half (<= 512 f32 PSUM bank)
NPAIR = HC // 2              # 48 channel pairs per core
PIT = 2                      # pairs per pipeline iteration
NIT = NPAIR // PIT           # 24 iterations
SP2 = 2 * S                  # 114 output partitions per pair

_cache = {}


def _dedupe_ldweights(nc):
    """bacc lowers every 16-bit matmul to an InstLdweights + InstMatmult pair.
    The PE serializes each load (~200ns) before its matmul.  The two
    half-column matmuls of a pair share the same stationary matrix, so drop
    the redundant reload: remove an InstLdweights whose weights AP equals the
    previous one on the PE stream, carrying its semaphore waits onto the next
    PE instruction.  The 64B ISA word has one wait slot, so only dedupe when
    the waits fit."""
    import concourse.mybir as mybir

    removed = 0
    for fn in nc.m.functions:
        for blk in fn.blocks:
            insts = list(blk.instructions)
            drop = set()
            last_key = None
            for i, inst in enumerate(insts):
                if getattr(inst, "engine", None) != mybir.EngineType.PE:
                    continue
                tn = type(inst).__name__
                if tn == "InstLdweights":
                    a = inst.ins[0]
                    key = (a.memref, a.offset, str(a.ap), str(a.dtype))
                    si = inst.sync_info
                    my_waits = list(si.on_wait) if si is not None else []
                    has_upd = si is not None and len(si.on_update) > 0
                    if key == last_key and not has_upd:
                        nxt = None
                        for j in range(i + 1, len(insts)):
                            if getattr(insts[j], "engine", None) == mybir.EngineType.PE:
                                nxt = insts[j]
                                break
                        if nxt is not None:
                            nsi = nxt.sync_info
                            n_waits = len(nsi.on_wait) if nsi is not None else 0
                            if n_waits + len(my_waits) <= 1:
                                if my_waits:
                                    if nsi is None:
                                        nxt.sync_info = mybir.SyncInfo(
                                            on_wait=my_waits, on_update=[]
                                        )
                                    else:
                                        nsi.on_wait = list(nsi.on_wait) + my_waits
                                drop.add(i)
                                removed += 1
                                continue
                    last_key = key
                elif tn == "InstMatmult":
                    pass  # non-self-loading; PE array state unchanged
                else:
                    last_key = None  # be conservative about other PE ops
            if drop:
                blk.instructions = [x for i, x in enumerate(insts) if i not in drop]
    return removed


def _build_nc():
    import concourse.bacc as bacc
    import concourse.mybir as mybir
    import concourse.tile as tile

    nc = bacc.Bacc("TRN2", target_bir_lowering=False, debug=False)
    f32 = mybir.dt.float32
    f16 = mybir.dt.float16

    x_d = nc.dram_tensor("x", [NIT, 128, PIT * COLS], f16, kind="ExternalInput").ap()
    w_d = nc.dram_tensor("w", [NIT, 128, PIT * SP2], f16, kind="ExternalInput").ap()
    b_d = nc.dram_tensor("b", [SP2, NPAIR], f32, kind="ExternalInput").ap()
    y_d = nc.dram_tensor("y", [NIT, SP2, PIT * COLS], f16, kind="ExternalOutput").ap()

    with tile.TileContext(nc) as tc:
        with (
            tc.tile_pool(name="cn", bufs=1) as cn,
            tc.tile_pool(name="wp", bufs=3) as wp,
            tc.tile_pool(name="xp", bufs=4) as xp,
            tc.tile_pool(name="op", bufs=4) as op_,
            tc.tile_pool(name="ps", bufs=2, space="PSUM") as psp,
        ):
            bt = cn.tile([SP2, NPAIR], f32)
            nc.sync.dma_start(bt[:], b_d[:])

            for it in range(NIT):
                wt = wp.tile([128, PIT, SP2], f16, tag="w")
                xt = xp.tile([128, PIT, COLS], f16, tag="x")
                if it == 0:
                    # first pair's x before w before second pair's x, so the
                    # pipeline fills as soon as ~270 KB has landed
                    nc.sync.dma_start(xt[:, 0], x_d[it][:, 0:COLS])
                    nc.sync.dma_start(wt[:], w_d[it])
                    nc.sync.dma_start(xt[:, 1], x_d[it][:, COLS:])
                else:
                    nc.sync.dma_start(wt[:], w_d[it])
                    nc.sync.dma_start(xt[:], x_d[it])
                ps = psp.tile([SP2, 2 * PIT, 512], f32, tag="ps")
                for q in range(PIT):
                    for j in range(2):
                        nc.tensor.matmul(
                            ps[:, 2 * q + j, 0:HB],
                            wt[:, q],
                            xt[:, q, j * HB : (j + 1) * HB],
                            start=True, stop=True,
                        )
                ot = op_.tile([SP2, PIT, 2, HB], f16, tag="o")
                for q in range(PIT):
                    pair = it * PIT + q
                    bias = bt[:, pair : pair + 1]
                    # alternate the PSUM->fp16+bias eviction between DVE and
                    # ACT (ACT also dispatches the out-DMAs); both engines
                    # stay under the DMA wall
                    if (2 * it + q) % 2 == 0:
                        nc.vector.tensor_scalar_add(
                            ot[:, q], ps[:, 2 * q : 2 * q + 2, 0:HB], bias
                        )
                    else:
                        nc.scalar.activation(
                            ot[:, q], ps[:, 2 * q : 2 * q + 2, 0:HB],
                            mybir.ActivationFunctionType.Identity,
                            bias=bias, scale=1.0,
                        )
                    if it == NIT - 1:
                        # split the last store so the drain is short
                        nc.scalar.dma_start(
                            y_d[it][:, q * COLS : (q + 1) * COLS], ot[:, q]
                        )
                if it < NIT - 1:
                    nc.scalar.dma_start(y_d[it], ot[:])

    nc.compile()
    n = _dedupe_ldweights(nc)
    if os.environ.get("KERNEL_VERBOSE"):
        print(f"deduped {n} ldweights")
    return nc


def _get_nc():
    if "nc" not in _cache:
        _cache["nc"] = _build_nc()
    return _cache["nc"]


def _install_ntff_hook():
    """antenv.axon_hooks is not shipped in this container; shim it so
    bass_utils can find the NTFF profile hook (trace=True path)."""
    import sys, types
    if "antenv.axon_hooks" in sys.modules:
        return
    mod = types.ModuleType("antenv.axon_hooks")
    mod._hook = None
    mod.set_axon_ntff_profile_hook = lambda h: setattr(mod, "_hook", h)
    mod.get_axon_ntff_profile_hook = lambda: mod._hook
    sys.modules["antenv.axon_hooks"] = mod
    try:
        from trn_agent_boot.trn_boot import _ntff_profile_via_ctypes
        mod._hook = _ntff_profile_via_ctypes("/opt/axon/libaxon_pjrt.so")
    except Exception:
        mod._hook = None


def kernel(x, rtg_w, rtg_b, obs_w, obs_b, act_w, act_b):
    from concourse import bass_utils

    x = np.asarray(x, dtype=np.float32)
    ws = np.stack([np.asarray(a, np.float32) for a in (rtg_w, obs_w, act_w)], 1)  # [H,3,W]
    bs = np.stack([np.asarray(a, np.float32) for a in (rtg_b, obs_b, act_b)], 1)  # [H,3]

    # staged input windows: xw[ch, m, (b, n)] = x[b, S*n + m - PADL, ch]
    xT = np.ascontiguousarray(x.transpose(2, 0, 1)).astype(np.float16)  # [H,B,T]
    xpad = np.zeros((H, B, XLEN), np.float16)
    xpad[:, :, PADL : PADL + T] = xT
    st = xpad.strides
    xw = as_strided(xpad, (H, B, NB, V), (st[0], st[1], S * st[2], st[2]))
    xw = np.ascontiguousarray(xw.transpose(0, 3, 1, 2)).reshape(H, V, COLS)

    # per-channel banded Toeplitz [V, S]: lh[ch, m, p] = w_{p%3}[ch, m-p]
    pidx = np.arange(S)
    lh = np.zeros((H, V, S), np.float32)
    for k in range(W):
        lh[:, pidx + k, pidx] = ws[:, pidx % 3, k]
    lh = lh.astype(np.float16)
    # pair block-diagonal stationaries [H/2, 128, 114]
    wpair = np.zeros((H // 2, 128, SP2), np.float16)
    wpair[:, :V, :S] = lh[0::2]
    wpair[:, V:, S:] = lh[1::2]
    # per-pair bias columns [114, H/2]: rows 0..56 ch_a, 57..113 ch_b
    bcol = bs[:, pidx % 3]                                  # [H, S]
    bpair = np.concatenate([bcol[0::2], bcol[1::2]], 1)     # [H/2, 114]
    bpair = np.ascontiguousarray(bpair.transpose(1, 0))     # [114, H/2]

    in_maps = []
    for c in range(NCORES):
        ch0 = c * HC
        xc = xw[ch0 : ch0 + HC].reshape(NPAIR, 128, COLS)   # pair rows stacked
        xc = xc.reshape(NIT, PIT, 128, COLS)
        xc = np.ascontiguousarray(xc.transpose(0, 2, 1, 3)).reshape(NIT, 128, PIT * COLS)
        wc = wpair[ch0 // 2 : ch0 // 2 + NPAIR].reshape(NIT, PIT, 128, SP2)
        wc = np.ascontiguousarray(wc.transpose(0, 2, 1, 3)).reshape(NIT, 128, PIT * SP2)
        bc = np.ascontiguousarray(bpair[:, ch0 // 2 : ch0 // 2 + NPAIR])
        in_maps.append({"x": xc, "w": wc, "b": bc})

    nc = _get_nc()
    trace = bool(int(os.environ.get("KERNEL_TRACE", "0")))
    if trace:
        _install_ntff_hook()
    res = bass_utils.run_bass_kernel_spmd(
        nc, in_maps, core_ids=list(range(NCORES)), trace=trace,
    )
    _cache["last_result"] = res

    out = np.empty((B, T, H), dtype=np.float32)
    for c in range(NCORES):
        y = res.results[c]["y"]                              # [NIT, SP2, PIT*COLS]
        y = y.reshape(NIT, 2, S, PIT, B, NB)                 # [it, half, p, q, b, n]
        y = y.transpose(4, 0, 3, 1, 5, 2)                    # [b, it, q, half, n, p]
        y = y.reshape(B, HC, NB * S)[:, :, :T]               # [b, ch, t]
        out[:, :, c * HC : (c + 1) * HC] = y.transpose(0, 2, 1).astype(np.float32)
    return out


# revision 8
# speedup vs baseline: 1.2672x; 1.2672x over previous
"""Depthwise causal conv1d (W=8) with 3 interleaved weight sets, on 8 TRN2 cores.

Reference computes r/o/a = depthwise_causal_conv(x, {rtg,obs,act}_{w,b}) and
interleaves out[:, t] = {r,o,a}[:, t] by t % 3.  Only the t%3-matching third of
each conv is needed, so total work is exactly one conv: for each output t,
out[b,t,h] = sum_k x[b, t-7+k, h] * w_{t%3}[h, k] + b_{t%3}[h].

Strategy (channel-parallel, 96 channels per core, banded-Toeplitz matmul,
two channels packed per stationary matrix):
  - time goes on the PE contraction axis.  For channel pair (ca, cb), one
    [128 x 108] fp16 stationary matrix is block-diagonal: rows 0..60 hold
    ca's banded Toeplitz T[m, p] = w_{p%3}[ca, m-p] (0 <= m-p < 8) over
    output columns 0..53, rows 61..121 hold cb's band over columns 54..107.
    S=54 outputs per block (54 % 3 == 0 keeps the t%3 phase pattern the
    same in every block; window V = S+7 = 61 rows).  Rows 122/123 of the
    moving tensor are a constant 1.0 and the matching stationary rows hold
    the per-channel bias columns, folding the bias into the matmul.
  - rhs = [128 x 912] stacks both channels' input windows for all
    912 = 16 batches x 57 blocks columns, so one weight load serves a
    pair's whole workload (halves the dense-w DMA vs 1 ch/stationary and
    computes 8 useful MACs per PE column-row vs 1 for a diag formulation).
  - each pair runs 2 matmuls (columns 0:456, 456:912 — PSUM bank limit is
    512 f32) that share one ldweights; a post-compile pass drops the
    redundant second weight load.
  - PSUM f32; per iteration (2 pairs) ONE ACT-or-DVE op downcast-evicts
    [108 x 4x456] to fp16 and ONE out-DMA stores it: single-writer tiles
    keep the tile-level dependency chain short (two writers per tile was
    measured to serialize evict->evict->store and stall PSUM reuse).
  - in-DMAs dispatched from the SP HWDGE, out-DMAs from the ACT HWDGE; big
    contiguous per-partition rows spread across all 16 DMA engines
    (~23 GB/s each), which is the binding resource for this kernel.
  - host pre/post stages the overlapped-window layout (fp16, unit-stride).
fp16 end-to-end rel err ~6e-4.
"""

import os
import numpy as np
from numpy.lib.stride_tricks import as_strided

B, T, H, W = 16, 3072, 768, 8
NCORES = 8
HC = H // NCORES             # 96 channels per core
S = 54                       # outputs per block (multiple of 3)
V = S + W - 1                # 61-row window per channel
NB = 57                      # blocks cover NB*S = 3078 >= T
PADL = W - 1                 # causal left zero-pad
XLEN = S * (NB - 1) + V      # 3085 padded time extent
COLS = B * NB                # 912 rhs columns per channel
HB = COLS // 2               # 456 matmul column half (<= 512 f32 PSUM bank)
SP2 = 2 * S                  # 108 output partitions per pair
NPAIR = HC // 2              # 48 channel pairs per core
PIT = 2                      # pairs per pipeline iteration
NIT = NPAIR // PIT           # 24 iterations

_cache = {}


def _dedupe_ldweights(nc):
    """bacc lowers every 16-bit matmul to an InstLdweights + InstMatmult pair.
    The PE serializes each load (~200ns) before its matmul.  The two
    half-column matmuls of a pair share the same stationary matrix, so drop
    the redundant reload: remove an InstLdweights whose weights AP equals the
    previous one on the PE stream, carrying its semaphore waits onto the next
    PE instruction.  The 64B ISA word has one wait slot, so only dedupe when
    the waits fit."""
    import concourse.mybir as mybir

    removed = 0
    for fn in nc.m.functions:
        for blk in fn.blocks:
            insts = list(blk.instructions)
            drop = set()
            last_key = None
            for i, inst in enumerate(insts):
                if getattr(inst, "engine", None) != mybir.EngineType.PE:
                    continue
                tn = type(inst).__name__
                if tn == "InstLdweights":
                    a = inst.ins[0]
                    key = (a.memref, a.offset, str(a.ap), str(a.dtype))
                    si = inst.sync_info
                    my_waits = list(si.on_wait) if si is not None else []
                    has_upd = si is not None and len(si.on_update) > 0
                    if key == last_key and not has_upd:
                        nxt = None
                        for j in range(i + 1, len(insts)):
                            if getattr(insts[j], "engine", None) == mybir.EngineType.PE:
                                nxt = insts[j]
                                break
                        if nxt is not None:
                            nsi = nxt.sync_info
                            n_waits = len(nsi.on_wait) if nsi is not None else 0
                            if n_waits + len(my_waits) <= 1:
                                if my_waits:
                                    if nsi is None:
                                        nxt.sync_info = mybir.SyncInfo(
                                            on_wait=my_waits, on_update=[]
                                        )
                                    else:
                                        nsi.on_wait = list(nsi.on_wait) + my_waits
                                drop.add(i)
                                removed += 1
                                continue
                    last_key = key
                elif tn == "InstMatmult":
                    pass  # non-self-loading; PE array state unchanged
                else:
                    last_key = None  # be conservative about other PE ops
            if drop:
                blk.instructions = [x for i, x in enumerate(insts) if i not in drop]
    return removed


def _build_nc():
    import concourse.bacc as bacc
    import concourse.mybir as mybir
    import concourse.tile as tile

    nc = bacc.Bacc("TRN2", target_bir_lowering=False, debug=False)
    f32 = mybir.dt.float32
    f16 = mybir.dt.float16

    x_d = nc.dram_tensor("x", [NIT, 128, PIT * COLS], f16, kind="ExternalInput").ap()
    w_d = nc.dram_tensor("w", [NIT, 128, PIT * SP2], f16, kind="ExternalInput").ap()
    y_d = nc.dram_tensor("y", [NIT, SP2, PIT * COLS], f16, kind="ExternalOutput").ap()

    with tile.TileContext(nc) as tc:
        with (
            tc.tile_pool(name="wp", bufs=3) as wp,
            tc.tile_pool(name="xp", bufs=4) as xp,
            tc.tile_pool(name="op", bufs=4) as op_,
            tc.tile_pool(name="ps", bufs=2, space="PSUM") as psp,
        ):
            for it in range(NIT):
                wt = wp.tile([128, PIT, SP2], f16, tag="w")
                xt = xp.tile([128, PIT, COLS], f16, tag="x")
                if it == 0:
                    # first pair's x before w before second pair's x, so the
                    # pipeline fills as soon as ~290 KB has landed
                    nc.sync.dma_start(xt[:, 0], x_d[it][:, 0:COLS])
                    nc.sync.dma_start(wt[:], w_d[it])
                    nc.sync.dma_start(xt[:, 1], x_d[it][:, COLS:])
                else:
                    nc.sync.dma_start(wt[:], w_d[it])
                    nc.sync.dma_start(xt[:], x_d[it])
                ps = psp.tile([SP2, 2 * PIT, 512], f32, tag="ps")
                for q in range(PIT):
                    for j in range(2):
                        nc.tensor.matmul(
                            ps[:, 2 * q + j, 0:HB],
                            wt[:, q],
                            xt[:, q, j * HB : (j + 1) * HB],
                            start=True, stop=True,
                        )
                ot = op_.tile([SP2, PIT, 2, HB], f16, tag="o")
                # one PSUM->fp16 eviction per iteration, ~3:2 DVE:ACT (ACT
                # also dispatches the out-DMAs); both stay under the DMA wall
                if it % 5 < 3:
                    nc.vector.tensor_scalar_mul(ot[:], ps[:, :, 0:HB], 1.0)
                else:
                    nc.scalar.copy(ot[:], ps[:, :, 0:HB])
                if it == NIT - 1:
                    # split the last store so the drain is short
                    for q in range(PIT):
                        nc.scalar.dma_start(
                            y_d[it][:, q * COLS : (q + 1) * COLS], ot[:, q]
                        )
                else:
                    nc.scalar.dma_start(y_d[it], ot[:])

    nc.compile()
    n = _dedupe_ldweights(nc)
    if os.environ.get("KERNEL_VERBOSE"):
        print(f"deduped {n} ldweights")
    return nc


def _get_nc():
    if "nc" not in _cache:
        _cache["nc"] = _build_nc()
    return _cache["nc"]


def _install_ntff_hook():
    """antenv.axon_hooks is not shipped in this container; shim it so
    bass_utils can find the NTFF profile hook (trace=True path)."""
    import sys, types
    if "antenv.axon_hooks" in sys.modules:
        return
    mod = types.ModuleType("antenv.axon_hooks")
    mod._hook = None
    mod.set_axon_ntff_profile_hook = lambda h: setattr(mod, "_hook", h)
    mod.get_axon_ntff_profile_hook = lambda: mod._hook
    sys.modules["antenv.axon_hooks"] = mod
    try:
        from trn_agent_boot.trn_boot import _ntff_profile_via_ctypes
        mod._hook = _ntff_profile_via_ctypes("/opt/axon/libaxon_pjrt.so")
    except Exception:
        mod._hook = None


def kernel(x, rtg_w, rtg_b, obs_w, obs_b, act_w, act_b):
    from concourse import bass_utils

    x = np.asarray(x, dtype=np.float32)
    ws = np.stack([np.asarray(a, np.float32) for a in (rtg_w, obs_w, act_w)], 1)  # [H,3,W]
    bs = np.stack([np.asarray(a, np.float32) for a in (rtg_b, obs_b, act_b)], 1)  # [H,3]

    # staged input windows: xw[ch, m, (b, n)] = x[b, S*n + m - PADL, ch]
    xT = np.ascontiguousarray(x.transpose(2, 0, 1)).astype(np.float16)  # [H,B,T]
    xpad = np.zeros((H, B, XLEN), np.float16)
    xpad[:, :, PADL : PADL + T] = xT
    st = xpad.strides
    xw = as_strided(xpad, (H, B, NB, V), (st[0], st[1], S * st[2], st[2]))
    xw = np.ascontiguousarray(xw.transpose(0, 3, 1, 2)).reshape(H, V, COLS)

    # per-channel banded Toeplitz [V, S]: lh[ch, m, p] = w_{p%3}[ch, m-p]
    pidx = np.arange(S)
    lh = np.zeros((H, V, S), np.float32)
    for k in range(W):
        lh[:, pidx + k, pidx] = ws[:, pidx % 3, k]
    lh = lh.astype(np.float16)
    bcol = bs[:, pidx % 3].astype(np.float16)               # [H, S]

    # pair block-diagonal stationaries [H/2, 128, 108] with bias rows 122/123
    HP = H // 2
    wpair = np.zeros((HP, 128, SP2), np.float16)
    wpair[:, :V, :S] = lh[0::2]
    wpair[:, V : 2 * V, S:] = lh[1::2]
    wpair[:, 122, :S] = bcol[0::2]
    wpair[:, 123, S:] = bcol[1::2]
    # paired moving windows [H/2, 128, 912] with the constant-1.0 bias rows
    xpair = np.zeros((HP, 128, COLS), np.float16)
    xpair[:, :V] = xw[0::2]
    xpair[:, V : 2 * V] = xw[1::2]
    xpair[:, 122:124] = 1.0

    in_maps = []
    for c in range(NCORES):
        p0 = c * NPAIR
        xc = xpair[p0 : p0 + NPAIR].reshape(NIT, PIT, 128, COLS)
        xc = np.ascontiguousarray(xc.transpose(0, 2, 1, 3)).reshape(NIT, 128, PIT * COLS)
        wc = wpair[p0 : p0 + NPAIR].reshape(NIT, PIT, 128, SP2)
        wc = np.ascontiguousarray(wc.transpose(0, 2, 1, 3)).reshape(NIT, 128, PIT * SP2)
        in_maps.append({"x": xc, "w": wc})

    nc = _get_nc()
    trace = bool(int(os.environ.get("KERNEL_TRACE", "0")))
    if trace:
        _install_ntff_hook()
    res = bass_utils.run_bass_kernel_spmd(
        nc, in_maps, core_ids=list(range(NCORES)), trace=trace,
    )
    _cache["last_result"] = res

    out = np.empty((B, T, H), dtype=np.float32)
    for c in range(NCORES):
        y = res.results[c]["y"]                              # [NIT, SP2, PIT*COLS]
        y = y.reshape(NIT, 2, S, PIT, B, NB)                 # [it, half, p, q, b, n]
        y = y.transpose(4, 0, 3, 1, 5, 2)                    # [b, it, q, half, n, p]
        y = y.reshape(B, HC, NB * S)[:, :, :T]               # [b, ch, t]
        out[:, :, c * HC : (c + 1) * HC] = y.transpose(0, 2, 1).astype(np.float32)
    return out


# revision 10
# speedup vs baseline: 1.3934x; 1.0996x over previous
"""Depthwise causal conv1d (W=8) with 3 interleaved weight sets, on 8 TRN2 cores.

Reference computes r/o/a = depthwise_causal_conv(x, {rtg,obs,act}_{w,b}) and
interleaves out[:, t] = {r,o,a}[:, t] by t % 3.  Only the t%3-matching third of
each conv is needed, so total work is exactly one conv: for each output t,
out[b,t,h] = sum_k x[b, t-7+k, h] * w_{t%3}[h, k] + b_{t%3}[h].

Strategy (channel-parallel, 96 channels per core, banded-Toeplitz matmul,
two channels packed per stationary matrix):
  - time goes on the PE contraction axis.  For channel pair (ca, cb), one
    [128 x 108] fp16 stationary matrix is block-diagonal: rows 0..60 hold
    ca's banded Toeplitz T[m, p] = w_{p%3}[ca, m-p] (0 <= m-p < 8) over
    output columns 0..53, rows 61..121 hold cb's band over columns 54..107.
    S=54 outputs per block (54 % 3 == 0 keeps the t%3 phase pattern the
    same in every block; window V = S+7 = 61 rows).  Rows 122/123 of the
    moving tensor are a constant 1.0 and the matching stationary rows hold
    the per-channel bias columns, folding the bias into the matmul.
  - rhs = [128 x 912] stacks both channels' input windows for all
    912 = 16 batches x 57 blocks columns, so one weight load serves a
    pair's whole workload (halves the dense-w DMA vs 1 ch/stationary and
    computes 8 useful MACs per PE column-row vs 1 for a diag formulation).
  - each pair runs 2 matmuls (columns 0:456, 456:912 — PSUM bank limit is
    512 f32) that share one ldweights; a post-compile pass drops the
    redundant second weight load.
  - PSUM f32; per iteration (2 pairs) ONE ACT-or-DVE op downcast-evicts
    [108 x 4x456] to fp16 and ONE out-DMA stores it: single-writer tiles
    keep the tile-level dependency chain short (two writers per tile was
    measured to serialize evict->evict->store and stall PSUM reuse).
  - in-DMAs dispatched from the SP HWDGE, out-DMAs from the ACT HWDGE; big
    contiguous per-partition rows spread across all 16 DMA engines
    (~23 GB/s each), which is the binding resource for this kernel.
  - host pre/post stages the overlapped-window layout (fp16, unit-stride).
fp16 end-to-end rel err ~6e-4.
"""

import os
import numpy as np
from numpy.lib.stride_tricks import as_strided

B, T, H, W = 16, 3072, 768, 8
NCORES = 8
HC = H // NCORES             # 96 channels per core
S = 54                       # outputs per block (multiple of 3)
V = S + W - 1                # 61-row window per channel
NB = 57                      # blocks cover NB*S = 3078 >= T
PADL = W - 1                 # causal left zero-pad
XLEN = S * (NB - 1) + V      # 3085 padded time extent
COLS = B * NB                # 912 rhs columns per channel
HB = COLS // 2               # 456 matmul column half (<= 512 f32 PSUM bank)
SP2 = 2 * S                  # 108 output partitions per pair
NPAIR = HC // 2              # 48 channel pairs per core
PIT = 4                      # pairs per pipeline iteration
NIT = NPAIR // PIT           # 12 iterations

_cache = {}


def _dedupe_ldweights(nc):
    """bacc lowers every 16-bit matmul to an InstLdweights + InstMatmult pair.
    The PE serializes each load (~200ns) before its matmul.  The two
    half-column matmuls of a pair share the same stationary matrix, so drop
    the redundant reload: remove an InstLdweights whose weights AP equals the
    previous one on the PE stream, carrying its semaphore waits onto the next
    PE instruction.  The 64B ISA word has one wait slot, so only dedupe when
    the waits fit."""
    import concourse.mybir as mybir

    removed = 0
    for fn in nc.m.functions:
        for blk in fn.blocks:
            insts = list(blk.instructions)
            drop = set()
            last_key = None
            for i, inst in enumerate(insts):
                if getattr(inst, "engine", None) != mybir.EngineType.PE:
                    continue
                tn = type(inst).__name__
                if tn == "InstLdweights":
                    a = inst.ins[0]
                    key = (a.memref, a.offset, str(a.ap), str(a.dtype))
                    si = inst.sync_info
                    my_waits = list(si.on_wait) if si is not None else []
                    has_upd = si is not None and len(si.on_update) > 0
                    if key == last_key and not has_upd:
                        nxt = None
                        for j in range(i + 1, len(insts)):
                            if getattr(insts[j], "engine", None) == mybir.EngineType.PE:
                                nxt = insts[j]
                                break
                        if nxt is not None:
                            nsi = nxt.sync_info
                            n_waits = len(nsi.on_wait) if nsi is not None else 0
                            if n_waits + len(my_waits) <= 1:
                                if my_waits:
                                    if nsi is None:
                                        nxt.sync_info = mybir.SyncInfo(
                                            on_wait=my_waits, on_update=[]
                                        )
                                    else:
                                        nsi.on_wait = list(nsi.on_wait) + my_waits
                                drop.add(i)
                                removed += 1
                                continue
                    last_key = key
                elif tn == "InstMatmult":
                    pass  # non-self-loading; PE array state unchanged
                else:
                    last_key = None  # be conservative about other PE ops
            if drop:
                blk.instructions = [x for i, x in enumerate(insts) if i not in drop]
    return removed


def _build_nc():
    import concourse.bacc as bacc
    import concourse.mybir as mybir
    import concourse.tile as tile

    nc = bacc.Bacc("TRN2", target_bir_lowering=False, debug=False)
    f32 = mybir.dt.float32
    f16 = mybir.dt.float16

    x_d = nc.dram_tensor("x", [NIT, 128, PIT * COLS], f16, kind="ExternalInput").ap()
    w_d = nc.dram_tensor("w", [NIT, 128, PIT * SP2], f16, kind="ExternalInput").ap()
    y_d = nc.dram_tensor("y", [NIT, SP2, PIT * COLS], f16, kind="ExternalOutput").ap()

    with tile.TileContext(nc) as tc:
        with (
            tc.tile_pool(name="wp", bufs=3) as wp,
            tc.tile_pool(name="xp", bufs=4) as xp,
            tc.tile_pool(name="op", bufs=4) as op_,
            tc.tile_pool(name="ps", bufs=2, space="PSUM") as psp,
        ):
            HP2 = PIT // 2
            for it in range(NIT):
                wt = wp.tile([128, PIT, SP2], f16, tag="w")
                xt = xp.tile([128, PIT, COLS], f16, tag="x")
                if it == 0:
                    # first pair's x before w before the rest, so the
                    # pipeline fills as soon as ~290 KB has landed
                    nc.sync.dma_start(xt[:, 0], x_d[it][:, 0:COLS])
                    nc.sync.dma_start(wt[:], w_d[it])
                    nc.sync.dma_start(xt[:, 1:], x_d[it][:, COLS:])
                else:
                    nc.sync.dma_start(wt[:], w_d[it])
                    nc.sync.dma_start(xt[:], x_d[it])
                # one 2-bank PSUM tile per pair (ring of 4) so eviction and
                # PSUM reuse proceed per-pair instead of per-iteration
                pss = []
                for q in range(PIT):
                    ps = psp.tile([SP2, 2, 512], f32, tag="ps")
                    for j in range(2):
                        nc.tensor.matmul(
                            ps[:, j, 0:HB],
                            wt[:, q],
                            xt[:, q, j * HB : (j + 1) * HB],
                            start=True, stop=True,
                        )
                    pss.append(ps)
                # evictions: pairs 0,1 on DVE -> ot_a; pairs 2,3 on ACT ->
                # ot_b.  Single-writer-engine tiles avoid the cross-engine
                # write-after-write serialization that stalls PSUM reuse.
                ota = op_.tile([SP2, HP2, 2, HB], f16, tag="oa")
                otb = op_.tile([SP2, HP2, 2, HB], f16, tag="ob")
                for q in range(PIT):
                    dst = ota[:, q] if q < HP2 else otb[:, q - HP2]
                    if q < HP2:
                        nc.vector.tensor_scalar_mul(dst, pss[q][:, :, 0:HB], 1.0)
                    else:
                        nc.scalar.copy(dst, pss[q][:, :, 0:HB])
                nc.scalar.dma_start(y_d[it][:, : HP2 * COLS], ota[:])
                nc.scalar.dma_start(y_d[it][:, HP2 * COLS :], otb[:])

    nc.compile()
    n = _dedupe_ldweights(nc)
    if os.environ.get("KERNEL_VERBOSE"):
        print(f"deduped {n} ldweights")
    return nc


def _get_nc():
    if "nc" not in _cache:
        _cache["nc"] = _build_nc()
    return _cache["nc"]


def _install_ntff_hook():
    """antenv.axon_hooks is not shipped in this container; shim it so
    bass_utils can find the NTFF profile hook (trace=True path)."""
    import sys, types
    if "antenv.axon_hooks" in sys.modules:
        return
    mod = types.ModuleType("antenv.axon_hooks")
    mod._hook = None
    mod.set_axon_ntff_profile_hook = lambda h: setattr(mod, "_hook", h)
    mod.get_axon_ntff_profile_hook = lambda: mod._hook
    sys.modules["antenv.axon_hooks"] = mod
    try:
        from trn_agent_boot.trn_boot import _ntff_profile_via_ctypes
        mod._hook = _ntff_profile_via_ctypes("/opt/axon/libaxon_pjrt.so")
    except Exception:
        mod._hook = None


def kernel(x, rtg_w, rtg_b, obs_w, obs_b, act_w, act_b):
    from concourse import bass_utils

    x = np.asarray(x, dtype=np.float32)
    ws = np.stack([np.asarray(a, np.float32) for a in (rtg_w, obs_w, act_w)], 1)  # [H,3,W]
    bs = np.stack([np.asarray(a, np.float32) for a in (rtg_b, obs_b, act_b)], 1)  # [H,3]

    # staged input windows: xw[ch, m, (b, n)] = x[b, S*n + m - PADL, ch]
    xT = np.ascontiguousarray(x.transpose(2, 0, 1)).astype(np.float16)  # [H,B,T]
    xpad = np.zeros((H, B, XLEN), np.float16)
    xpad[:, :, PADL : PADL + T] = xT
    st = xpad.strides
    xw = as_strided(xpad, (H, B, NB, V), (st[0], st[1], S * st[2], st[2]))
    xw = np.ascontiguousarray(xw.transpose(0, 3, 1, 2)).reshape(H, V, COLS)

    # per-channel banded Toeplitz [V, S]: lh[ch, m, p] = w_{p%3}[ch, m-p]
    pidx = np.arange(S)
    lh = np.zeros((H, V, S), np.float32)
    for k in range(W):
        lh[:, pidx + k, pidx] = ws[:, pidx % 3, k]
    lh = lh.astype(np.float16)
    bcol = bs[:, pidx % 3].astype(np.float16)               # [H, S]

    # pair block-diagonal stationaries [H/2, 128, 108] with bias rows 122/123
    HP = H // 2
    wpair = np.zeros((HP, 128, SP2), np.float16)
    wpair[:, :V, :S] = lh[0::2]
    wpair[:, V : 2 * V, S:] = lh[1::2]
    wpair[:, 122, :S] = bcol[0::2]
    wpair[:, 123, S:] = bcol[1::2]
    # paired moving windows [H/2, 128, 912] with the constant-1.0 bias rows
    xpair = np.zeros((HP, 128, COLS), np.float16)
    xpair[:, :V] = xw[0::2]
    xpair[:, V : 2 * V] = xw[1::2]
    xpair[:, 122:124] = 1.0

    in_maps = []
    for c in range(NCORES):
        p0 = c * NPAIR
        xc = xpair[p0 : p0 + NPAIR].reshape(NIT, PIT, 128, COLS)
        xc = np.ascontiguousarray(xc.transpose(0, 2, 1, 3)).reshape(NIT, 128, PIT * COLS)
        wc = wpair[p0 : p0 + NPAIR].reshape(NIT, PIT, 128, SP2)
        wc = np.ascontiguousarray(wc.transpose(0, 2, 1, 3)).reshape(NIT, 128, PIT * SP2)
        in_maps.append({"x": xc, "w": wc})

    nc = _get_nc()
    trace = bool(int(os.environ.get("KERNEL_TRACE", "0")))
    if trace:
        _install_ntff_hook()
    res = bass_utils.run_bass_kernel_spmd(
        nc, in_maps, core_ids=list(range(NCORES)), trace=trace,
    )
    _cache["last_result"] = res

    out = np.empty((B, T, H), dtype=np.float32)
    for c in range(NCORES):
        y = res.results[c]["y"]                              # [NIT, SP2, PIT*COLS]
        y = y.reshape(NIT, 2, S, PIT, B, NB)                 # [it, half, p, q, b, n]
        y = y.transpose(4, 0, 3, 1, 5, 2)                    # [b, it, q, half, n, p]
        y = y.reshape(B, HC, NB * S)[:, :, :T]               # [b, ch, t]
        out[:, :, c * HC : (c + 1) * HC] = y.transpose(0, 2, 1).astype(np.float32)
    return out
